# revision 17
# baseline (speedup 1.0000x reference)
"""Trainium2 Bass kernel for EditOuterAttention (dense transformer cross-attention).

Reference computation (BS=2, LX=LY=2048, D=1024, H=16, DK=64):
    q = x @ Wq + bq ; k = y @ Wk + bk ; v = y @ Wv + bv     (per batch)
    scores = q @ k^T / sqrt(DK) + mask
    out = (softmax(scores) @ v) @ Wo + bo

Sharding: 8 cores = 2 (batch) x 4 (head groups of 4 heads / 256 dims).
Per core (batch b, head-group g):
    - column-parallel QKV projections over the 256-dim head slice
    - attention for 4 heads -> normalized attention output AO [256, 2048] (bf16)
    - AllGather(AO) across the 4 cores of batch b (per 512-col sx block, so
      the collective overlaps the remaining attention compute)
    - column-parallel output projection with the core's Wo column slice
      -> out rows [256, 2048] of O^T; host reassembles + transposes.

Dataflow notes:
    - All matmuls run in bf16 (inputs cast host-side / on-chip) with fp32
      PSUM accumulation; measured end-to-end relative error ~5e-3.
    - Q^T/K^T are produced directly in [head_dim, seq] layout by using the
      weight matrix as the stationary operand (out = W^T @ x^T).
    - Scores are computed transposed (S^T[sy, sx]) so the exp'd tiles can be
      used directly as the moving operand of the AV matmul.
    - The softmax denominator comes for free from a ones-column appended to
      the stationary [V | 1] operand of the AV matmul; normalization happens
      on the [64, sx] AV output via reciprocal + a K=1 bf16 broadcast matmul
      whose operands sit at base partitions 0/32/64/96 (no row copies).
    - Blocks are sx-major ((sx block, head), head inner) so each sx block's
      AO completes early and its AllGather overlaps the next block's
      attention; the matching output projection is emitted two blocks later.
    - Score and AV matmuls are emission-interleaved so the Act engine's exp
      (1.1us per [128,1024] tile) keeps pace with the PE.
    - 1/sqrt(DK) is folded into the Exp activation's scale; zero biases and
      zero mask (the common case) compile out entirely.  Nonzero bq/bk are
      applied on-chip, nonzero bv/bo are exact host-side post-corrections,
      and a nonzero mask multiplies exp(mask)^T into the exp'd score tiles.
"""

import numpy as np
import ml_dtypes

import concourse.bass as bass
import concourse.bacc as bacc
import concourse.tile as tile
import concourse.mybir as mybir
from concourse.bass_utils import run_bass_kernel_spmd

BS, LX, LY, D, H, DK = 2, 2048, 2048, 1024, 16, 64
NCORES = 8
NGRP = 4            # head groups (tensor-parallel)
HD = H * DK // NGRP  # 256 head dims per core
NH = H // NGRP       # 4 heads per core
SXB = 512            # sx block
NSXB = LX // SXB     # 4
NSYT = LY // 128     # 16 sy tiles
NDC = D // 128       # 8 contraction chunks
OUT_ROWS = D // NGRP  # 256 rows of O^T per core (column-parallel oproj)

F32 = mybir.dt.float32
BF16 = mybir.dt.bfloat16
EXPF = mybir.ActivationFunctionType.Exp

_compiled = {}


def _build(has_qk_bias: bool, has_mask: bool, n_cores: int = NCORES,
           with_collective: bool = True):
    nc = bacc.Bacc("TRN2", target_bir_lowering=False, debug=False,
                   num_devices=n_cores)

    xT = nc.dram_tensor("xT", [D, LX], BF16, kind="ExternalInput")
    yT = nc.dram_tensor("yT", [D, LY], BF16, kind="ExternalInput")
    wq = nc.dram_tensor("wq", [D, HD], BF16, kind="ExternalInput")
    wk = nc.dram_tensor("wk", [D, HD], BF16, kind="ExternalInput")
    wv = nc.dram_tensor("wv", [D, HD], BF16, kind="ExternalInput")
    # column slice of Wo: [D, OUT_ROWS]
    wo = nc.dram_tensor("wo", [D, OUT_ROWS], BF16, kind="ExternalInput")
    if has_qk_bias:
        bq = nc.dram_tensor("bq", [HD], F32, kind="ExternalInput")
        bk = nc.dram_tensor("bk", [HD], F32, kind="ExternalInput")
    if has_mask:
        em = nc.dram_tensor("em", [LY, LX], BF16, kind="ExternalInput")
    if with_collective:
        out_ext = nc.dram_tensor("out", [OUT_ROWS, LX], F32, kind="ExternalOutput")
    else:
        # sim path: the core's normalized attention output AO [256, 2048]
        out_ext = nc.dram_tensor("out", [HD, LX], BF16, kind="ExternalOutput")

    st_bufs = 2 if has_mask else 3

    with tile.TileContext(nc) as tc:
        with (
            tc.tile_pool(name="persist", bufs=1) as pp,
            tc.tile_pool(name="st", bufs=st_bufs) as stp,
            tc.tile_pool(name="ostage", bufs=2) as osp,
            tc.tile_pool(name="small", bufs=3) as smp,
            tc.tile_pool(name="scp", bufs=2, space="PSUM") as scp,
            tc.tile_pool(name="mmp", bufs=2, space="PSUM") as mmp,
            tc.tile_pool(name="avp", bufs=2, space="PSUM") as avp,
            tc.tile_pool(name="dram", bufs=1, space="DRAM") as drp,
        ):
            # ---- static inputs -> SBUF --------------------------------
            # interleaved chunk-wise so the first matmul (wq c0 + xT c0)
            # isn't gated on the whole input load
            wq_sb = pp.tile([128, NDC * HD], BF16, tag="wq")
            wk_sb = pp.tile([128, NDC * HD], BF16, tag="wk")
            wv_sb = pp.tile([128, NDC * HD], BF16, tag="wv")
            xT_sb = pp.tile([128, NDC * LX], BF16, tag="xT")
            yT_sb = pp.tile([128, NDC * LY], BF16, tag="yT")
            for d in range(NDC):
                nc.sync.dma_start(out=wq_sb[:, d * HD:(d + 1) * HD],
                                  in_=wq[d * 128:(d + 1) * 128, :])
                nc.sync.dma_start(out=xT_sb[:, d * LX:(d + 1) * LX],
                                  in_=xT[d * 128:(d + 1) * 128, :])
            for d in range(NDC):
                nc.sync.dma_start(out=wk_sb[:, d * HD:(d + 1) * HD],
                                  in_=wk[d * 128:(d + 1) * 128, :])
                nc.sync.dma_start(out=yT_sb[:, d * LY:(d + 1) * LY],
                                  in_=yT[d * 128:(d + 1) * 128, :])
                nc.sync.dma_start(out=wv_sb[:, d * HD:(d + 1) * HD],
                                  in_=wv[d * 128:(d + 1) * 128, :])
            wo_sb = pp.tile([128, NDC * OUT_ROWS], BF16, tag="wo")
            for c in range(NDC):
                nc.sync.dma_start(out=wo_sb[:, c * OUT_ROWS:(c + 1) * OUT_ROWS],
                                  in_=wo[c * 128:(c + 1) * 128, :])
            if has_qk_bias:
                bq_sb = pp.tile([128, 2], F32, tag="bq")
                bk_sb = pp.tile([128, 2], F32, tag="bk")
                nc.sync.dma_start(out=bq_sb[:], in_=bq.ap().rearrange("(t p) -> p t", p=128))
                nc.sync.dma_start(out=bk_sb[:], in_=bk.ap().rearrange("(t p) -> p t", p=128))

            # bf16 ones rows at partitions 0/32/64/96 for the K=1
            # normalize-broadcast matmuls (operand base partition = 32*h)
            ones_bf = pp.tile([128, 64], BF16, tag="ones")
            nc.vector.memset(ones_bf[:], 1.0)

            # ---- Q^T / K^T projections: out [hd, seq] -----------------
            # Q^T = Wq^T @ x^T via lhsT = Wq chunk, rhs = x^T chunk.
            QT_sb = pp.tile([128, 2 * LX], BF16, tag="QT")
            KT_sb = pp.tile([128, 2 * LY], BF16, tag="KT")

            def emit_qk_proj(t):             # 128-dim slice of head dims
                # two 512-col chains share one [128, 1024] psum tile from the
                # (idle at this point) score ring; one wide copy-out each
                for (w_sb, src_sb, dst_sb, bias_name) in (
                    (wq_sb, xT_sb, QT_sb, "bq"),
                    (wk_sb, yT_sb, KT_sb, "bk"),
                ):
                    for sbp in range(NSXB // 2):
                        ps = scp.tile([128, 1024], F32, tag="sc")
                        for d in range(NDC):
                            for j in range(2):
                                nc.tensor.matmul(
                                    ps[:, j * SXB:(j + 1) * SXB],
                                    lhsT=w_sb[:, d * HD + t * 128: d * HD + (t + 1) * 128],
                                    rhs=src_sb[:, d * LX + sbp * 1024 + j * SXB:
                                               d * LX + sbp * 1024 + j * SXB + SXB],
                                    start=(d == 0), stop=(d == NDC - 1),
                                    skip_group_check=True)
                        dst = dst_sb[:, t * LX + sbp * 1024:
                                     t * LX + sbp * 1024 + 1024]
                        if has_qk_bias:
                            b_sb = bq_sb if bias_name == "bq" else bk_sb
                            nc.vector.tensor_scalar_add(dst, ps[:], b_sb[:, t:t + 1])
                        else:
                            nc.vector.tensor_copy(dst, ps[:])

            # ---- V projection: out [seq, hd] interleaved with ones ----
            # V1 layout per sy tile: [128, NH*65] = 4 x (64 v-dims + ones col)
            V1_sb = pp.tile([128, NSYT * NH * 65], BF16, tag="V1")

            def emit_v_proj(st):
                ps = mmp.tile([128, HD], F32, tag="mm")
                for d in range(NDC):
                    nc.tensor.matmul(
                        ps[:],
                        lhsT=yT_sb[:, d * LY + st * 128: d * LY + st * 128 + 128],
                        rhs=wv_sb[:, d * HD:(d + 1) * HD],
                        start=(d == 0), stop=(d == NDC - 1))
                dst = V1_sb[:, st * NH * 65:(st + 1) * NH * 65] \
                    .rearrange("p (h c) -> p h c", c=65)[:, :, 0:64]
                nc.vector.tensor_copy(dst, ps[:].rearrange("p (h c) -> p h c", c=64))

            # ---- mask (rare path): exp(mask)^T blocks per sx block ----
            em_blocks = {}

            def load_mask_block(sb):
                mb = stp.tile([128, NSYT * SXB], BF16, tag="mask",
                              bufs=1 if has_mask else 1)
                for st in range(NSYT):
                    nc.sync.dma_start(
                        out=mb[:, st * SXB:(st + 1) * SXB],
                        in_=em[st * 128:(st + 1) * 128, sb * SXB:(sb + 1) * SXB])
                em_blocks[sb] = mb

            # ---- attention blocks: sx-major (sb, h), h inner ----------
            blocks = [(sb, h) for sb in range(NSXB) for h in range(NH)]
            st_tiles = {}
            av_tiles = {}
            grp_state = {}

            def emit_score_pair(i, ST, s2):
                sb, h = blocks[i]
                ht, hr = h // 2, (h % 2) * 64
                ps = scp.tile([128, 1024], F32, tag="sc")
                for j in range(2):
                    st = 2 * s2 + j
                    nc.tensor.matmul(
                        ps[:, j * SXB:(j + 1) * SXB],
                        lhsT=KT_sb[hr:hr + 64, ht * LY + st * 128: ht * LY + st * 128 + 128],
                        rhs=QT_sb[hr:hr + 64, ht * LX + sb * SXB: ht * LX + sb * SXB + SXB],
                        start=True, stop=True)
                dst = ST[:, s2 * 1024:(s2 + 1) * 1024]
                nc.scalar.activation(dst, ps[:], EXPF, scale=1.0 / (DK ** 0.5))
                if has_mask:
                    mb = em_blocks[blocks[i][0]]
                    nc.vector.tensor_mul(dst, dst, mb[:, s2 * 1024:(s2 + 1) * 1024])

            def emit_av_mm(j, st):
                sb, h = blocks[j]
                if st == 0:
                    av_tiles[j] = avp.tile([65, SXB], F32, tag="av",
                                           name=f"av{j}")
                pav = av_tiles[j]
                ST = st_tiles[j]
                nc.tensor.matmul(
                    pav[:],
                    lhsT=V1_sb[:, st * NH * 65 + h * 65: st * NH * 65 + h * 65 + 65],
                    rhs=ST[:, st * SXB:(st + 1) * SXB],
                    start=(st == 0), stop=(st == NSYT - 1),
                    skip_group_check=True)

            LASTG = len(blocks) // NH - 1

            def emit_av_post(j):
                # den row + unnormalized AV -> SBUF; free the psum + ST
                sb, h = blocks[j]
                pav = av_tiles.pop(j)
                st_tiles.pop(j)
                g = j // NH
                if h == 0:
                    dt_ = smp.tile([128, SXB], F32, tag="dt", bufs=2,
                                   name=f"dt{g}")
                    # reciprocal reads the full tile; fill unused rows
                    nc.vector.memset(dt_[:], 1.0)
                    grp_state[g] = (dt_, {})
                dt_, unns = grp_state[g]
                if g == LASTG and h == NH - 1:
                    # last head of the last group goes to its own den tile so
                    # heads 0-2 can normalize under the final AV chain
                    dtb = smp.tile([128, SXB], F32, tag="dtb", bufs=1,
                                   name="dtb")
                    nc.vector.memset(dtb[:], 1.0)
                    nc.vector.tensor_copy(dtb[0:1, :], pav[64:65, :])
                    grp_state[g] = (dt_, unns, dtb)
                else:
                    nc.vector.tensor_copy(dt_[32 * h:32 * h + 1, :],
                                          pav[64:65, :])
                un = smp.tile([64, SXB], BF16, tag="un", bufs=6, name=f"un{j}")
                nc.vector.tensor_copy(un[:], pav[0:64, :])
                unns[h] = un

            def emit_block(i):
                # scores for block i interleaved with the AV chain of block
                # i-1 (keeps Act exp off the PE critical path)
                sb, h = blocks[i]
                if has_mask and h == 0:
                    load_mask_block(sb)
                ST = stp.tile([128, NSYT * SXB], BF16, tag="st")
                st_tiles[i] = ST
                for s2 in range(NSYT // 2):
                    emit_score_pair(i, ST, s2)
                    if i > 0:
                        emit_av_mm(i - 1, 2 * s2)
                        emit_av_mm(i - 1, 2 * s2 + 1)
                if i > 0:
                    emit_av_post(i - 1)

            ao_tiles = {}

            def emit_pbc_mul(g, h, rr_ap, ones_ap, ao):
                # normalize-broadcast (K=1 matmul) + multiply for one head;
                # psum from the mmp ring (idle during attention/tail)
                pbc = mmp.tile([64, SXB], F32, tag="mm", name=f"pbc{g}_{h}")
                nc.tensor.matmul(pbc[:], lhsT=ones_ap, rhs=rr_ap,
                                 start=True, stop=True)
                unns = grp_state[g][1]
                nc.vector.tensor_mul(
                    ao[(h % 2) * 64:(h % 2) * 64 + 64,
                       (h // 2) * SXB:(h // 2) * SXB + SXB],
                    unns[h][:], pbc[:])

            def emit_normalize(g):
                dt_, unns = grp_state[g]
                rr = smp.tile([128, SXB], BF16, tag="rr", bufs=2, name=f"rr{g}")
                r = smp.tile([128, SXB], F32, tag="r", bufs=2, name=f"r{g}")
                nc.vector.reciprocal(r[:], dt_[:])
                nc.vector.tensor_copy(rr[:], r[:])
                ao = osp.tile([128, 2 * SXB], BF16, tag="ao", name=f"ao{g}")
                ao_tiles[g] = ao
                # matmul operand base partition must be 0/32/64; head 3's
                # reciprocal row (partition 96) needs one row copy
                rr3 = smp.tile([1, SXB], BF16, tag="rr3", bufs=2, name=f"rr3_{g}")
                nc.vector.tensor_copy(rr3[:], rr[96:97, :])
                for h in range(NH):
                    if h < 3:
                        emit_pbc_mul(g, h, rr[32 * h:32 * h + 1, :],
                                     ones_bf[32 * h:32 * h + 1, :], ao)
                    else:
                        emit_pbc_mul(g, h, rr3[:], ones_bf[0:1, :], ao)
                grp_state.pop(g)

            # ---- AllGather of AO + column-parallel output projection --
            groups = [[g * NGRP + r for r in range(NGRP)]
                      for g in range(max(n_cores // NGRP, 1))]
            if with_collective:
                ao_stage = [drp.tile([HD, SXB], BF16, tag=f"aostg{g}",
                                     space="DRAM", name=f"aostg{g}")
                            for g in range(NSXB)]
                ag_out = [drp.tile([D, SXB], BF16, tag=f"agout{g}",
                                   space="DRAM", name=f"agout{g}")
                          for g in range(NSXB)]
                # tiny dummy collective at kernel start absorbs the ~11.5us
                # first-trigger CC setup cost while the input DMAs run
                ccw_in = drp.tile([128, 4], BF16, tag="ccwi", space="DRAM",
                                  name="ccwi")
                ccw_out = drp.tile([512, 4], BF16, tag="ccwo", space="DRAM",
                                   name="ccwo")
                nc.gpsimd.collective_compute(
                    "AllGather", mybir.AluOpType.bypass,
                    replica_groups=groups,
                    ins=[ccw_in.opt()], outs=[ccw_out.opt()])

            def ao_dma_half(g, ao, half):
                # one half (two heads) of the AO block -> staging rows
                tgt = (ao_stage[g][half * 128:(half + 1) * 128, :]
                       if with_collective else
                       out_ext[half * 128:(half + 1) * 128,
                               g * SXB:(g + 1) * SXB])
                nc.sync.dma_start(out=tgt,
                                  in_=ao[:, half * SXB:(half + 1) * SXB])

            def emit_ag(g):
                if with_collective:
                    nc.gpsimd.collective_compute(
                        "AllGather", mybir.AluOpType.bypass,
                        replica_groups=groups,
                        ins=[ao_stage[g].opt()],
                        outs=[ag_out[g].opt()])

            def emit_ao_ag(g):
                ao = ao_tiles.pop(g)
                ao_dma_half(g, ao, 0)
                ao_dma_half(g, ao, 1)
                emit_ag(g)

            def emit_oproj(g):
                if not with_collective:
                    return
                agin = smp.tile([128, NDC * SXB], BF16, tag="agin", bufs=2,
                                name=f"agin{g}")
                # single 3D-AP DMA for the gathered AO block
                nc.sync.dma_start(
                    out=agin[:].rearrange("p (c f) -> p c f", c=NDC),
                    in_=ag_out[g][:].rearrange("(c p) f -> p c f", p=128))
                ost = osp.tile([128, 2 * SXB], F32, tag="ost")
                for t in range(2):
                    po = mmp.tile([128, SXB], F32, tag="mm")
                    for c in range(NDC):
                        nc.tensor.matmul(
                            po[:],
                            lhsT=wo_sb[:, c * OUT_ROWS + t * 128:
                                       c * OUT_ROWS + (t + 1) * 128],
                            rhs=agin[:, c * SXB:(c + 1) * SXB],
                            start=(c == 0), stop=(c == NDC - 1))
                    nc.vector.tensor_copy(ost[:, t * SXB:(t + 1) * SXB], po[:])
                nc.sync.dma_start(
                    out=out_ext[:, g * SXB:(g + 1) * SXB]
                    .rearrange("(c p) f -> p c f", p=128),
                    in_=ost[:].rearrange("p (c f) -> p c f", c=2))

            # ---- schedule ---------------------------------------------
            emit_qk_proj(0)
            # block 0 scores interleaved with the V projection as PE filler
            # so Act's exp backlog never stalls the PE
            sb0, h0 = blocks[0]
            if has_mask:
                load_mask_block(sb0)
            ST0 = stp.tile([128, NSYT * SXB], BF16, tag="st")
            st_tiles[0] = ST0
            ones_cols = V1_sb[:].rearrange("p (t h c) -> p t h c",
                                           t=NSYT, c=65)[:, :, :, 64:65]
            nc.vector.memset(ones_cols, 1.0)
            for s2 in range(NSYT // 2):
                emit_score_pair(0, ST0, s2)
                emit_v_proj(2 * s2)
                emit_v_proj(2 * s2 + 1)
            emit_qk_proj(1)

            # all output projections are deferred to the tail: mid-stream the
            # PE runs pure attention (no collective-dependent matmuls), and at
            # the tail oproj(0..2) fill the final AllGather's latency window
            for i in range(1, len(blocks)):
                emit_block(i)
                if i % NH == 0:
                    g = i // NH - 1
                    emit_normalize(g)
                    emit_ao_ag(g)

            # ---- tail: split normalize of the last group ---------------
            # heads 0-2 normalize under the final AV chain via the Act
            # engine's 1/x = exp(-ln(x)) (same act table as exp, no swap);
            # head 3 runs the shortest possible chain after the last AV.
            last = len(blocks) - 1
            g = LASTG
            dt_, unns = grp_state[g][0], grp_state[g][1]
            rr_a = smp.tile([128, SXB], BF16, tag="rr", bufs=2, name="rr_a")
            lnd = smp.tile([128, SXB], F32, tag="lnd", bufs=1, name="lnd")
            ao = osp.tile([128, 2 * SXB], BF16, tag="ao", name=f"ao{g}")
            ao_tiles[g] = ao
            nc.scalar.activation(lnd[:], dt_[:],
                                 mybir.ActivationFunctionType.Ln)
            nc.scalar.activation(rr_a[:], lnd[:], EXPF, scale=-1.0)
            for st in range(NSYT):
                emit_av_mm(last, st)
                # heads 0-2 pbc+mul interleave into the AV chain once the
                # Act engine has drained the last score exps
                if st in (8, 11, 14):
                    h = (st - 8) // 3
                    emit_pbc_mul(g, h, rr_a[32 * h:32 * h + 1, :],
                                 ones_bf[32 * h:32 * h + 1, :], ao)
                if st == 15:
                    ao_dma_half(g, ao, 0)   # heads 0,1 staged early
            emit_av_post(last)
            dtb = grp_state[g][2]
            lnb = smp.tile([128, SXB], F32, tag="lnd", bufs=1, name="lnb")
            rr_b = smp.tile([128, SXB], BF16, tag="rr", bufs=2, name="rr_b")
            nc.scalar.activation(lnb[:], dtb[:],
                                 mybir.ActivationFunctionType.Ln)
            nc.scalar.activation(rr_b[:], lnb[:], EXPF, scale=-1.0)
            emit_pbc_mul(g, NH - 1, rr_b[0:1, :], ones_bf[0:1, :], ao)
            grp_state.pop(g)
            ao_tiles.pop(g)
            ao_dma_half(g, ao, 1)
            emit_ag(g)
            for gg in range(NSXB):
                emit_oproj(gg)

    nc.compile()
    return nc


def _get_compiled(has_qk_bias: bool, has_mask: bool):
    key = (has_qk_bias, has_mask)
    if key not in _compiled:
        _compiled[key] = _build(has_qk_bias, has_mask)
    return _compiled[key]


def _prep_inputs(x, y, mask, Wq, bq, Wk, bk, Wv, bv, Wo, bo,
                 has_qk_bias, has_mask):
    bf = ml_dtypes.bfloat16
    xT = [np.ascontiguousarray(x[b].T).astype(bf) for b in range(BS)]
    yT = [np.ascontiguousarray(y[b].T).astype(bf) for b in range(BS)]
    if has_mask:
        em = [np.ascontiguousarray(np.exp(mask[b, 0]).T).astype(bf)
              for b in range(BS)]
    in_maps = []
    for c in range(NCORES):
        b, g = c // NGRP, c % NGRP
        sl = slice(g * HD, (g + 1) * HD)
        osl = slice(g * OUT_ROWS, (g + 1) * OUT_ROWS)
        m = {
            "xT": xT[b], "yT": yT[b],
            "wq": np.ascontiguousarray(Wq[:, sl]).astype(bf),
            "wk": np.ascontiguousarray(Wk[:, sl]).astype(bf),
            "wv": np.ascontiguousarray(Wv[:, sl]).astype(bf),
            "wo": np.ascontiguousarray(Wo[:, osl]).astype(bf),
        }
        if has_qk_bias:
            m["bq"] = np.ascontiguousarray(bq[sl]).astype(np.float32)
            m["bk"] = np.ascontiguousarray(bk[sl]).astype(np.float32)
        if has_mask:
            m["em"] = em[b]
        in_maps.append(m)
    return in_maps


def kernel(x, y, mask, Wq, bq, Wk, bk, Wv, bv, Wo, bo):
    x = np.asarray(x, np.float32)
    y = np.asarray(y, np.float32)
    mask = np.asarray(mask, np.float32)
    has_qk_bias = bool(np.any(bq) or np.any(bk))
    has_mask = bool(np.any(mask))
    nc = _get_compiled(has_qk_bias, has_mask)
    in_maps = _prep_inputs(x, y, mask, Wq, bq, Wk, bk, Wv, bv, Wo, bo,
                           has_qk_bias, has_mask)
    res = run_bass_kernel_spmd(nc, in_maps, list(range(NCORES)))
    out = np.empty((BS, LX, D), np.float32)
    for b in range(BS):
        OT = np.concatenate(
            [res.results[b * NGRP + r]["out"] for r in range(NGRP)], axis=0)
        out[b] = OT.T
    bv = np.asarray(bv, np.float32)
    bo = np.asarray(bo, np.float32)
    if bv.any() or bo.any():
        # softmax rows sum to 1 => v-bias passes through attention exactly.
        out += (bv @ np.asarray(Wo, np.float32) + bo)[None, None, :]
    return out
